# revision 31
# baseline (speedup 1.0000x reference)
"""Trainium2 Bass/Tile kernel for GroupNorm + MultiHeadAttention + proj + residual.

Reference computation (per batch b):
    xf  = x[b] reshaped (C, T=H*W)
    xn  = GroupNorm32(xf) * norm_w + norm_b          (per-channel affine)
    qkv = qkv_w @ xn + qkv_b                         (3C, T)
    per head h (8 heads, hd=64):
        scores = (q*s)^T (k*s), s = hd**-0.25        (T, T)
        P = softmax(scores, axis=-1)
        h_out = P @ v^T  -> (hd, T)
    y   = proj_w @ h + proj_b + xf                   (C, T)

Distribution: pure data parallel over batch: 16 batches / 8 cores = 2 per core.
No collectives; each core runs the same NEFF on its own batch shard.

Layout strategy (per batch, all fp32):
  - x, xn stored as 4 SBUF tiles (128ch, 1024t); channels on partitions.
  - GroupNorm stats via bn_stats/bn_aggr per channel, 16-channel group
    aggregation and group->channel broadcast via small PE matmuls with
    host-precomputed masks.
  - Q, K computed natural (o on partitions); V computed directly transposed
    (t on partitions) by using xn as lhsT, so no on-chip transposes anywhere.
  - scores computed transposed, (s on partitions, t free): lhsT=k_h, rhs=q_h.
    exp on ScalarE (PSUM->SBUF); row-sums obtained for free by appending a
    ones-column to the PV lhsT ([v^T | 1] -> PSUM row 64 = colsum).
  - softmax normalization deferred: 1/sums via reciprocal_approx_fast, then
    broadcast (head -> 64 channels) with one tiny K=8 selector matmul per
    c-tile, applied during H normalization on VectorE.
  - proj consumes normalized H (c on partitions); bias+residual fused into
    the PSUM evacuation (x pre-biased in place).
"""

import numpy as np

import concourse.bass as bass
import concourse.mybir as mybir
import concourse.tile as tile
from concourse import bacc

F32 = mybir.dt.float32
F32R = mybir.dt.float32r
AF = mybir.ActivationFunctionType
OP = mybir.AluOpType

B, C, HH, WW = 16, 512, 32, 32
T = HH * WW            # 1024
NH, HD = 8, 64         # heads, head dim
N_CORES = 8
BPC = B // N_CORES     # batches per core = 2
CT = C // 128          # 4 channel tiles
ST = T // 128          # 8 s-chunks / t-tiles
GROUPS = 32
GS = C // GROUPS       # 16 channels per group
GPT = 128 // GS        # 8 groups per 128-channel tile
EPS = 1e-5
SCALE = float(HD) ** -0.25


def _build_body(ctx, tc, d):
    nc = tc.nc
    assert BPC == 2  # the emission schedule below is hand-pipelined for 2

    const = ctx.enter_context(tc.tile_pool(name="const", bufs=1))
    sb = ctx.enter_context(tc.tile_pool(name="sb", bufs=1))
    ps = ctx.enter_context(tc.tile_pool(name="ps", space="PSUM", bufs=1))

    # ---- x loads first (they gate groupnorm), then consts by need -------
    S = [dict() for _ in range(BPC)]
    for b in range(BPC):
        S[b]["x"] = []
        for k in range(CT):
            xk = sb.tile([128, T], F32, name=f"x{b}_{k}", tag=f"x{k}", bufs=2)
            eng = nc.sync if (k + b) % 2 == 0 else nc.gpsimd
            for half in range(2):
                eng.dma_start(
                    out=xk[:, half * 512:(half + 1) * 512],
                    in_=d["x"][b, k * 128:(k + 1) * 128, half * 512:(half + 1) * 512],
                )
            S[b]["x"].append(xk)
        S[b]["qk"] = {}
        S[b]["vT"] = []
        S[b]["ht"] = []
        S[b]["r_bf"] = {}

    # small consts needed by groupnorm / qkv evac
    gmask = const.tile([128, GPT], F32, name="gmask")
    nc.gpsimd.dma_start(out=gmask, in_=d["gmask"])
    bmask = const.tile([GPT, 128], F32, name="bmask")
    nc.gpsimd.dma_start(out=bmask, in_=d["bmask"])
    nwc = const.tile([128, CT], F32, name="nwc")
    nc.gpsimd.dma_start(out=nwc, in_=d["nw_cols"])
    nbc = const.tile([128, CT], F32, name="nbc")
    nc.gpsimd.dma_start(out=nbc, in_=d["nb_cols"])
    qkb = const.tile([128, 2 * CT], F32, name="qkb")
    nc.gpsimd.dma_start(out=qkb, in_=d["qk_bias_cols"])
    zeros = const.tile([128, 1], F32, name="zeros")
    nc.vector.memset(zeros, 0.0)
    ones8 = const.tile([128, NH, 1], F32, name="ones8")
    nc.vector.memset(ones8, 1.0)

    # qkv weights (needed ~15us in)
    qkv_wT = []
    for k in range(CT):
        w1 = const.tile([128, 3 * C], F32R, name=f"qkv_wT{k}")
        eng = nc.gpsimd if k % 2 == 0 else nc.sync
        eng.dma_start(out=w1, in_=d["qkv_wT"][k * 128:(k + 1) * 128, :])
        qkv_wT.append(w1)
    vbias = const.tile([128, C], F32, name="vbias")
    nc.gpsimd.dma_start(out=vbias, in_=d["v_bias_bc"])
    sel = const.tile([2, C], mybir.dt.bfloat16, name="sel")
    nc.gpsimd.dma_start(out=sel, in_=d["sel"])
    sel_r = []
    for j in range(2):
        sj = const.tile([1, C], mybir.dt.bfloat16, name=f"sel_r{j}")
        nc.gpsimd.dma_start(out=sj, in_=d["sel"][j:j + 1, :])
        sel_r.append(sj)

    # proj weights (needed late)
    proj_wT = []
    for k in range(CT):
        w2 = const.tile([128, C], F32R, name=f"proj_wT{k}")
        nc.sync.dma_start(out=w2, in_=d["proj_wT"][k * 128:(k + 1) * 128, :])
        proj_wT.append(w2)
    pbc = const.tile([128, CT], F32, name="pbc")
    nc.gpsimd.dma_start(out=pbc, in_=d["pb_cols"])

    # ---- emitters -------------------------------------------------------
    def emit_gn(b):
        """GroupNorm stats + per-channel affine -> xn tiles."""
        x = S[b]["x"]
        ge = sb.tile([GPT, CT, 2], F32, name=f"ge{b}", tag="ge", bufs=2)
        for k in range(CT):
            st = sb.tile([128, 2, 6], F32, name=f"st{b}_{k}", tag="st", bufs=2)
            nc.vector.bn_stats(out=st[:, 0, :], in_=x[k][:, 0:512])
            nc.vector.bn_stats(out=st[:, 1, :], in_=x[k][:, 512:1024])
            mv = sb.tile([128, 2], F32, name=f"mv{b}_{k}", tag="mv", bufs=2)
            nc.vector.bn_aggr(out=mv, in_=st)
            s2 = sb.tile([128, 2], F32, name=f"s2{b}_{k}", tag="s2", bufs=2)
            nc.vector.tensor_copy(out=s2[:, 0:1], in_=mv[:, 0:1])
            nc.vector.tensor_mul(s2[:, 1:2], mv[:, 0:1], mv[:, 0:1])
            nc.vector.tensor_add(s2[:, 1:2], s2[:, 1:2], mv[:, 1:2])
            gp = ps.tile([GPT, 2], F32, name=f"gp{b}_{k}", tag="mm_ps", bufs=2)
            nc.tensor.matmul(gp, gmask, s2, start=True, stop=True)
            nc.vector.tensor_copy(out=ge[:, k, :], in_=gp)

        gstats = sb.tile([GPT, CT, 2], F32, name=f"gstats{b}", tag="gstats", bufs=2)
        gvar = sb.tile([GPT, CT], F32, name=f"gvar{b}", tag="gvar", bufs=2)
        nc.vector.tensor_mul(gvar, ge[:, :, 0], ge[:, :, 0])
        nc.vector.tensor_sub(gvar, ge[:, :, 1], gvar)
        nc.vector.tensor_scalar_add(gvar, gvar, EPS)
        nc.scalar.activation(out=gvar, in_=gvar, func=AF.Sqrt, bias=zeros[0:GPT, :])
        nc.vector.reciprocal(out=gstats[:, :, 1], in_=gvar)
        nc.vector.tensor_copy(out=gstats[:, :, 0], in_=ge[:, :, 0])

        xn = []
        for k in range(CT):
            cps = ps.tile([128, 2], F32, name=f"cps{b}_{k}", tag="mm_ps", bufs=2)
            nc.tensor.matmul(cps, bmask, gstats[:, k, :], start=True, stop=True)
            A = sb.tile([128, 1], F32, name=f"A{b}_{k}", tag=f"A{k}", bufs=2)
            Bc = sb.tile([128, 1], F32, name=f"B{b}_{k}", tag=f"B{k}", bufs=2)
            nc.vector.tensor_mul(A, cps[:, 1:2], nwc[:, k:k + 1])
            nc.vector.tensor_mul(Bc, cps[:, 0:1], A)
            nc.vector.tensor_sub(Bc, nbc[:, k:k + 1], Bc)
            xnk = sb.tile([128, T], F32R, name=f"xn{b}_{k}", tag=f"xn{k}", bufs=2)
            nc.vector.tensor_scalar(
                out=xnk, in0=x[k], scalar1=A, scalar2=Bc, op0=OP.mult, op1=OP.add
            )
            xn.append(xnk)
        S[b]["xn"] = xn

    def emit_qk(b, m):
        """One 128-row output tile of Q (m<4) or K (m>=4)."""
        xn = S[b]["xn"]
        dst = sb.tile([128, T], F32R, name=f"qk{b}_{m}", tag=f"qk{m}", bufs=1)
        for n in range(2):
            qk_ps = ps.tile([128, 512], F32, name=f"qk_ps{b}_{m}_{n}", tag="mm_ps", bufs=2)
            for k in range(CT):
                nc.tensor.matmul(
                    qk_ps,
                    qkv_wT[k][:, m * 128:(m + 1) * 128],
                    xn[k][:, n * 512:(n + 1) * 512],
                    start=(k == 0),
                    stop=(k == CT - 1),
                )
            nc.vector.tensor_scalar(
                out=dst[:, n * 512:(n + 1) * 512], in0=qk_ps,
                scalar1=qkb[:, m:m + 1], scalar2=None, op0=OP.add
            )
        S[b]["qk"][m] = dst

    def emit_vt(b, mts=None):
        """V^T tiles (t on partitions), interleaved (128, NH, HD+1) with a
        ones column for the PV row-sum trick."""
        xn = S[b]["xn"]
        if not S[b]["vT"]:
            S[b]["vT"] = [
                sb.tile([128, NH, HD + 1], F32R, name=f"vT{b}_{mt}",
                        tag=f"vT{mt}", bufs=1)
                for mt in range(ST)
            ]
        for mt in (mts if mts is not None else range(ST)):
            v_ps = ps.tile([128, 512], F32, name=f"v_ps{b}_{mt}", tag="mm_ps", bufs=2)
            for k in range(CT):
                nc.tensor.matmul(
                    v_ps,
                    xn[k][:, mt * 128:(mt + 1) * 128],
                    qkv_wT[k][:, 2 * C:3 * C],
                    start=(k == 0),
                    stop=(k == CT - 1),
                )
            vt_t = S[b]["vT"][mt]
            nc.vector.tensor_tensor(
                out=vt_t[:, :, 0:HD],
                in0=v_ps.rearrange("p (h d) -> p h d", h=NH),
                in1=vbias.rearrange("p (h d) -> p h d", h=NH),
                op=OP.add,
            )
            nc.vector.tensor_copy(out=vt_t[:, :, HD:HD + 1], in_=ones8)

    def emit_head_chunks(b, h, scs, first):
        """scores^T -> exp -> PV accumulate, for a subset of s-chunks."""
        qt = S[b]["qk"][h // 2]
        kt = S[b]["qk"][CT + h // 2]
        vT = S[b]["vT"]
        qh = qt[(h % 2) * 64:(h % 2) * 64 + 64, :]
        kh = kt[(h % 2) * 64:(h % 2) * 64 + 64, :]
        if first:
            S[b][f"pv{h}"] = ps.tile([HD + 1, T], F32, name=f"pv{b}_{h}",
                                     tag="pv_ps", bufs=1)
        pv_ps = S[b][f"pv{h}"]
        for j, sc in enumerate(scs):
            sT_ps = ps.tile([128, T], F32, name=f"sT{b}_{h}_{sc}", tag="sT_ps", bufs=2)
            pT = sb.tile([128, T], F32R, name=f"pT{b}_{h}_{sc}", tag="pT", bufs=2)
            for n in range(2):
                nc.tensor.matmul(
                    sT_ps[:, n * 512:(n + 1) * 512],
                    kh[:, sc * 128:(sc + 1) * 128],
                    qh[:, n * 512:(n + 1) * 512],
                    start=True,
                    stop=True,
                )
            nc.scalar.activation(out=pT, in_=sT_ps, func=AF.Exp, bias=zeros)
            for n in range(2):
                nc.tensor.matmul(
                    pv_ps[:, n * 512:(n + 1) * 512],
                    vT[sc][:, h, :],
                    pT[:, n * 512:(n + 1) * 512],
                    start=(first and j == 0),
                    stop=False,
                    skip_group_check=True,
                )

    def emit_head(b, h, scs=None, first=True):
        """Attention for one head (optionally the remaining chunk subset)."""
        if h == 0 and not S[b]["ht"]:
            S[b]["ht"] = [
                sb.tile([128, T], F32R, name=f"h{b}_{k}", tag=f"h{k}", bufs=1)
                for k in range(CT)
            ]
        ht = S[b]["ht"]
        if h % 2 == 0:
            S[b]["sums"] = sb.tile([2, T], F32, name=f"sums{b}_{h // 2}",
                                   tag="sums", bufs=2)
        sums = S[b]["sums"]
        emit_head_chunks(b, h, scs if scs is not None else range(ST), first)
        pv_ps = S[b][f"pv{h}"]
        # unnormalized head output + row sums
        fast = h >= NH - 2   # last pair: keep recip off the sums-DMA path
        stg = sb.tile([1, T], F32, name=f"stg{b}_{h}", tag="stg", bufs=2)
        nc.vector.tensor_copy(out=stg, in_=pv_ps[HD:HD + 1, :])
        if fast:
            S[b].setdefault("stg_fast", {})[h % 2] = stg
        else:
            nc.gpsimd.dma_start(out=sums[h % 2:h % 2 + 1, :], in_=stg)
        nc.vector.tensor_copy(
            out=ht[h // 2][(h % 2) * 64:(h % 2) * 64 + 64, :],
            in_=pv_ps[0:HD, :],
        )
        if h % 2 == 1:
            hp = h // 2
            if fast:
                rbfs = []
                for j in range(2):
                    rpj = sb.tile([1, T], F32, name=f"r{b}_{hp}_{j}", tag="r", bufs=1)
                    nc.vector.reciprocal_approx_fast(
                        out=rpj, in_=S[b]["stg_fast"][j]
                    )
                    rbfj = sb.tile([1, T], mybir.dt.bfloat16,
                                   name=f"rbf{b}_{hp}_{j}", tag="r_bf", bufs=5)
                    nc.vector.tensor_copy(out=rbfj, in_=rpj)
                    rbfs.append(rbfj)
                S[b]["r_bf"][hp] = tuple(rbfs)
            else:
                rp = sb.tile([2, T], F32, name=f"r{b}_{hp}", tag="r", bufs=1)
                nc.vector.reciprocal_approx_fast(out=rp, in_=sums)
                rp_bf = sb.tile([2, T], mybir.dt.bfloat16, name=f"rbf{b}_{hp}",
                                tag="r_bf", bufs=5)
                nc.vector.tensor_copy(out=rp_bf, in_=rp)
                S[b]["r_bf"][hp] = rp_bf

    def emit_norm(b, k):
        """ht[k] *= broadcast(1/sums of pair k)."""
        ht = S[b]["ht"]
        rbf = S[b]["r_bf"][k]
        for n in range(2):
            rbc_ps = ps.tile([128, 512], F32, name=f"rbc{b}_{k}_{n}",
                             tag="mm_ps", bufs=2)
            if isinstance(rbf, tuple):
                for j in range(2):
                    nc.tensor.matmul(
                        rbc_ps,
                        sel_r[j][:, k * 128:(k + 1) * 128],
                        rbf[j][:, n * 512:(n + 1) * 512],
                        start=(j == 0),
                        stop=(j == 1),
                    )
            else:
                nc.tensor.matmul(
                    rbc_ps,
                    sel[:, k * 128:(k + 1) * 128],
                    rbf[:, n * 512:(n + 1) * 512],
                    start=True,
                    stop=True,
                )
            nc.vector.tensor_mul(
                ht[k][:, n * 512:(n + 1) * 512],
                ht[k][:, n * 512:(n + 1) * 512],
                rbc_ps,
            )

    def emit_proj(b, m):
        """proj output tile m + bias + residual + store."""
        ht = S[b]["ht"]
        y = sb.tile([128, T], F32, name=f"y{b}_{m}", tag=f"xn{m}", bufs=2)
        for n in range(2):
            pj_ps = ps.tile([128, 512], F32, name=f"pj{b}_{m}_{n}",
                            tag="mm_ps", bufs=2)
            for k in range(CT):
                nc.tensor.matmul(
                    pj_ps,
                    proj_wT[k][:, m * 128:(m + 1) * 128],
                    ht[k][:, n * 512:(n + 1) * 512],
                    start=(k == 0),
                    stop=(k == CT - 1),
                )
            nc.vector.scalar_tensor_tensor(
                out=y[:, n * 512:(n + 1) * 512], in0=pj_ps,
                scalar=pbc[:, m:m + 1],
                in1=S[b]["x"][m][:, n * 512:(n + 1) * 512],
                op0=OP.add, op1=OP.add,
            )
        eng = nc.sync if m % 2 == 0 else nc.gpsimd
        eng.dma_start(out=d["out"][b, m * 128:(m + 1) * 128, :], in_=y)

    # ---- hand-pipelined emission schedule -------------------------------
    emit_gn(0)
    emit_qk(0, 0); emit_qk(0, 4)
    emit_vt(0, [0, 1, 2])
    emit_gn(1)                      # Sqrt lands on ACT before the first Exp
    emit_head_chunks(0, 0, [0, 1, 2], first=True)
    emit_vt(0, [3, 4, 5, 6, 7])
    emit_head(0, 0, scs=[3, 4, 5, 6, 7], first=False)
    emit_qk(0, 1); emit_qk(0, 5)
    emit_head(0, 1)
    emit_qk(0, 2); emit_qk(0, 6)
    emit_head(0, 2)
    emit_qk(0, 3); emit_qk(0, 7)
    emit_head(0, 3)
    emit_norm(0, 0)
    emit_qk(1, 0); emit_qk(1, 4)
    emit_head(0, 4)
    emit_qk(1, 1); emit_qk(1, 5)
    emit_head(0, 5)
    emit_norm(0, 1)
    emit_qk(1, 2); emit_qk(1, 6)
    emit_head(0, 6)
    emit_qk(1, 3); emit_qk(1, 7)
    emit_head(0, 7)
    emit_norm(0, 2)
    emit_vt(1)
    emit_norm(0, 3)
    emit_head(1, 0)
    emit_proj(0, 0); emit_proj(0, 1)
    emit_head(1, 1)
    emit_proj(0, 2); emit_proj(0, 3)
    emit_head(1, 2)
    emit_norm(1, 0)
    emit_head(1, 3)
    emit_head(1, 4)
    emit_norm(1, 1)
    emit_head(1, 5)
    emit_head(1, 6)
    emit_norm(1, 2)
    emit_head(1, 7)
    emit_norm(1, 3)
    for m in range(CT):
        emit_proj(1, m)


def build_nc():
    nc = bacc.Bacc("TRN2")
    d = {}
    d["x"] = nc.dram_tensor("x", [BPC, C, T], F32, kind="ExternalInput")[:]
    d["qkv_wT"] = nc.dram_tensor("qkv_wT", [C, 3 * C], F32R, kind="ExternalInput")[:]
    d["proj_wT"] = nc.dram_tensor("proj_wT", [C, C], F32R, kind="ExternalInput")[:]
    d["qk_bias_cols"] = nc.dram_tensor(
        "qk_bias_cols", [128, 2 * CT], F32, kind="ExternalInput"
    )[:]
    d["v_bias_bc"] = nc.dram_tensor("v_bias_bc", [128, C], F32, kind="ExternalInput")[:]
    d["nw_cols"] = nc.dram_tensor("nw_cols", [128, CT], F32, kind="ExternalInput")[:]
    d["nb_cols"] = nc.dram_tensor("nb_cols", [128, CT], F32, kind="ExternalInput")[:]
    d["pb_cols"] = nc.dram_tensor("pb_cols", [128, CT], F32, kind="ExternalInput")[:]
    d["gmask"] = nc.dram_tensor("gmask", [128, GPT], F32, kind="ExternalInput")[:]
    d["bmask"] = nc.dram_tensor("bmask", [GPT, 128], F32, kind="ExternalInput")[:]
    d["sel"] = nc.dram_tensor("sel", [2, C], mybir.dt.bfloat16, kind="ExternalInput")[:]
    d["out"] = nc.dram_tensor("out", [BPC, C, T], F32, kind="ExternalOutput")[:]

    from contextlib import ExitStack

    with tile.TileContext(nc) as tc:
        with ExitStack() as ctx:
            _build_body(ctx, tc, d)
    nc.finalize()
    return nc


def host_inputs(x, norm_w, norm_b, qkv_w, qkv_b, proj_w, proj_b):
    """Host-side constant preprocessing (numpy, cheap)."""
    f = np.float32
    # Reference splits qkv per head: after reshape (B*nh, 3*hd, T), head h's
    # q/k/v are original rows [192h,192h+64), [192h+64,192h+128),
    # [192h+128,192h+192). Permute rows so the kernel sees q (all heads,
    # head-major), then k, then v.
    perm = np.concatenate([
        np.concatenate([np.arange(3 * HD * h + j * HD, 3 * HD * h + (j + 1) * HD)
                        for h in range(NH)])
        for j in range(3)
    ])
    qkv_w = np.asarray(qkv_w, f)[perm].copy()
    qkv_b = np.asarray(qkv_b, f)[perm].copy()
    # fold the q/k scale (hd**-0.25) into the weights and biases
    qkv_w[: 2 * C] *= f(SCALE)
    qkv_b[: 2 * C] *= f(SCALE)

    consts = {
        "qkv_wT": np.ascontiguousarray(qkv_w.T),
        "proj_wT": np.ascontiguousarray(np.asarray(proj_w, f).T),
        "qk_bias_cols": np.ascontiguousarray(
            qkv_b[: 2 * C].reshape(2 * CT, 128).T
        ),
        "v_bias_bc": np.ascontiguousarray(
            np.broadcast_to(qkv_b[2 * C:], (128, C))
        ),
        "nw_cols": np.ascontiguousarray(np.asarray(norm_w, f).reshape(CT, 128).T),
        "nb_cols": np.ascontiguousarray(np.asarray(norm_b, f).reshape(CT, 128).T),
        "pb_cols": np.ascontiguousarray(np.asarray(proj_b, f).reshape(CT, 128).T),
    }
    gmask = np.zeros((128, GPT), f)
    for p in range(128):
        gmask[p, p // GS] = 1.0 / GS
    consts["gmask"] = gmask
    consts["bmask"] = np.ascontiguousarray((gmask.T > 0).astype(f))
    sel_ = np.zeros((2, C), f)
    for c in range(C):
        sel_[(c // HD) % 2, c] = 1.0
    import ml_dtypes
    consts["sel"] = sel_.astype(ml_dtypes.bfloat16)

    xs = np.ascontiguousarray(np.asarray(x, f).reshape(N_CORES, BPC, C, T))
    return xs, consts


_NC_CACHE = None


def kernel(x, norm_w, norm_b, qkv_w, qkv_b, proj_w, proj_b, num_heads=8, **_):
    from concourse.bass_utils import run_bass_kernel_spmd

    assert int(num_heads) == NH
    global _NC_CACHE
    if _NC_CACHE is None:
        _NC_CACHE = build_nc()
    nc = _NC_CACHE

    xs, consts = host_inputs(x, norm_w, norm_b, qkv_w, qkv_b, proj_w, proj_b)
    in_maps = [{"x": xs[i], **consts} for i in range(N_CORES)]
    res = run_bass_kernel_spmd(nc, in_maps, core_ids=list(range(N_CORES)))
    out = np.stack([res.results[i]["out"] for i in range(N_CORES)])
    return out.reshape(B, C, HH, WW)
